# revision 38
# baseline (speedup 1.0000x reference)
"""Local windowed attention (window=128, look_backward=1, RoPE) on 8 TRN2 cores.

Sharding: data-parallel over batch (128 -> 16 per core).

Host prep (layout/dtype/embedding-preprocessing only -- all of the attention
itself, i.e. logits, causal-masked softmax and the weighted sum over values,
runs on device):
  * RoPE rotation applied to q,k on the host using global-position angles
    (rotation-invariance of dot products makes this exactly equivalent to the
    reference's window-relative angles); q,k uploaded pre-transposed d-major
    [64, N] bf16 with two batches stacked on the 128 SBUF partitions.
  * v blocked per window with a ones column appended (the softmax denominator
    then falls out of MM2's PSUM accumulation).

Device:
  * MM1 (logits^T, j-major): stationary k^T_w [64,128], moving [q^T_w|q^T_{w+1}]
    [64,256] -> one matmul per window produces the strip [cur_w | back_{w+1}].
    The two batches of a pair live on partition halves 0:63 / 64:127, so their
    K=64 matmuls carry tile_position (0,0) / (64,0): interleaving them in issue
    order runs them CONCURRENTLY on disjoint PE row-groups and overlaps
    LDWEIGHTS of one half with the matmul of the other.
  * 1/sqrt(D) scale folded into the Exp activation's scale parameter; one exp
    call covers a [128,1024] PSUM chunk holding 2 windows of BOTH batches.
  * causal mask: ONE strided tensor_tensor per chunk over its 4 cur blocks
    (outer dim 4 x stride 256) against a broadcast [128,128] 0/1 mask; MM2
    groups fire as soon as the chunk holding their last window is masked
    (h1 staggered one chunk later), so the whole pair pipelines at chunk
    granularity and the scalar engine (the bottleneck at ~136us/core) stays
    ~100% busy mid-run.
  * MM2 accumulates [back|cur] x v into PSUM groups of up to 7 windows per
    bank (the final windows in 2-window groups to shorten the tail); ONE
    strided reciprocal + ONE broadcast tensor_tensor per group normalizes and
    evacuates straight to bf16 SBUF.
  * Output written blocked [pos-in-window, (window, d)] bf16; host
    inverse-permutes and upcasts.

Measured on trn2 (8 cores, core-0 NTFF profile): 153.6us HW exec vs 368.6us
for the previous baseline (2.40x); rel err 5.3e-3 (tolerance 2e-2).
"""

import sys

sys.path.insert(0, "/opt/trn_rl_repo")

import numpy as np
import ml_dtypes

import concourse.bass as bass
import concourse.bacc as bacc
import concourse.mybir as mybir
import concourse.tile as tile
from concourse.bass_utils import run_bass_kernel_spmd

B, N, D, W = 128, 4096, 64, 128
NCORES = 8
BL = B // NCORES          # 16 batches per core
NP = BL // 2              # 8 batch-pairs per core
NW = N // W               # 32 windows
THETA = 10000.0
NCH = NW // 2             # S-chunks per pair (2 windows x 2 batches each)
# MM2 window groups (start, len): 7-window groups fill a PSUM bank; the final
# windows are split into 2-window groups so little MM2 work trails the last exp
GRP = [(0, 7), (7, 7), (14, 7), (21, 7), (28, 2), (30, 2)]

BF16 = mybir.dt.bfloat16
F32 = mybir.dt.float32
NPBF16 = ml_dtypes.bfloat16

_CACHE = {}


def _ecur(w, h):
    """Column of window w's cur block (batch-half h) in the per-pair E tile."""
    return 1024 * (w // 2) + 512 * h + 256 * (w % 2)


def _build_program():
    nc = bacc.Bacc(None, target_bir_lowering=False, debug=False)
    qt = nc.dram_tensor("qt", [NP * 128, N], BF16, kind="ExternalInput")
    kt = nc.dram_tensor("kt", [NP * 128, N], BF16, kind="ExternalInput")
    vb = nc.dram_tensor("vb", [BL * 128, NW * 65], BF16, kind="ExternalInput")
    m01 = nc.dram_tensor("m01", [128, 128], BF16, kind="ExternalInput")
    outb = nc.dram_tensor("outb", [BL * 128, NW * D], BF16, kind="ExternalOutput")

    with tile.TileContext(nc) as tc:
        with (
            tc.tile_pool(name="const", bufs=1) as constp,
            tc.tile_pool(name="io", bufs=2) as iop,
            tc.tile_pool(name="ep", bufs=2) as ep,
            tc.tile_pool(name="rp", bufs=2) as rp,
            tc.tile_pool(name="ob", bufs=2) as obp,
            tc.tile_pool(name="ps", bufs=3, space="PSUM") as psp,
            tc.tile_pool(name="po", bufs=1, space="PSUM") as pop,
        ):
            m_t = constp.tile([128, 128], BF16, tag="m01")

            # MM2 groups whose trigger chunk is >= DEFER_AT are issued during
            # the NEXT pair's early chunks so the scalar engine never waits
            # for them at the pair boundary.
            DEFER_AT = 99  # deferral measured slower; disabled
            pending = []  # list of (mm2_group_fn,) carried into the next pair

            for p in range(NP):
                q_ = iop.tile([128, N], BF16, tag="q")
                k_ = iop.tile([128, N], BF16, tag="k")
                if p == 0:
                    # split first loads so chunk-0 matmuls start early; the
                    # mask constant, exp-table warmup and the rest follow
                    nc.sync.dma_start(out=q_[:, 0:512], in_=qt[0:128, 0:512])
                    nc.scalar.dma_start(
                        out=k_[:, 0:512], in_=kt[0:128, 0:512]
                    )
                    nc.sync.dma_start(
                        out=q_[:, 512:1024], in_=qt[0:128, 512:1024]
                    )
                    nc.scalar.dma_start(
                        out=k_[:, 512:1024], in_=kt[0:128, 512:1024]
                    )
                    nc.sync.dma_start(out=m_t[:], in_=m01[:])
                    # dependency-free dummy exp: pulls the ~2.7us
                    # ACT_TABLE_LOAD into the DMA head (input is garbage)
                    warm = constp.tile([128, 1], BF16, tag="warm")
                    warm_in = constp.tile([128, 1], BF16, tag="warm_in")
                    nc.vector.memset(warm_in[:], 0.0)
                    nc.scalar.activation(
                        warm[:], warm_in[:],
                        mybir.ActivationFunctionType.Exp,
                    )
                    nc.sync.dma_start(
                        out=q_[:, 1024:N], in_=qt[0:128, 1024:N]
                    )
                    nc.scalar.dma_start(
                        out=k_[:, 1024:N], in_=kt[0:128, 1024:N]
                    )
                else:
                    nc.sync.dma_start(
                        out=q_[:], in_=qt[p * 128:(p + 1) * 128, :]
                    )
                    nc.sync.dma_start(
                        out=k_[:], in_=kt[p * 128:(p + 1) * 128, :]
                    )
                v0 = iop.tile([128, NW * 65], BF16, tag="v0")
                v1 = iop.tile([128, NW * 65], BF16, tag="v1")
                nc.sync.dma_start(
                    out=v0[:], in_=vb[2 * p * 128:(2 * p + 1) * 128, :]
                )
                nc.sync.dma_start(
                    out=v1[:], in_=vb[(2 * p + 1) * 128:(2 * p + 2) * 128, :]
                )

                # E layout per pair: chunk c holds windows {2c, 2c+1} for both
                # batch halves: [h0: cur|back|cur|back (512) | h1: same (512)]
                eh = ep.tile([128, NCH * 1024], BF16, tag="eh")
                osb = {}
                for h in range(2):
                    osb[h] = obp.tile([128, NW * D], BF16, tag=f"osb{h}", name=f"osb{h}")

                def mm2_group(h, gi, p=p, eh=eh, v0=v0, v1=v1, osb=osb):
                    v_ = v0 if h == 0 else v1
                    g0, gl = GRP[gi]
                    O = pop.tile([128, 512], F32, tag=f"O{h}", name=f"O{h}")
                    for j in range(gl):
                        w = g0 + j
                        if w == 0:
                            nc.tensor.matmul(
                                O[:, 0:65],
                                lhsT=eh[:, _ecur(0, h): _ecur(0, h) + 128],
                                rhs=v_[:, 0:65],
                                start=True, stop=True,
                            )
                        else:
                            bk = _ecur(w - 1, h) + 128
                            nc.tensor.matmul(
                                O[:, j * 65:(j + 1) * 65],
                                lhsT=eh[:, bk: bk + 128],
                                rhs=v_[:, (w - 1) * 65: w * 65],
                                start=True, stop=False,
                            )
                            cu = _ecur(w, h)
                            nc.tensor.matmul(
                                O[:, j * 65:(j + 1) * 65],
                                lhsT=eh[:, cu: cu + 128],
                                rhs=v_[:, w * 65:(w + 1) * 65],
                                start=False, stop=True,
                            )
                    r = rp.tile([128, 8], F32, tag=f"r{h}", name=f"r{h}")
                    ogrp = O[:, 0: gl * 65].rearrange("p (w c) -> p w c", c=65)
                    nc.vector.reciprocal(r[:, 0:gl], ogrp[:, :, 64])
                    nc.vector.tensor_mul(
                        osb[h][:, g0 * D: (g0 + gl) * D].rearrange(
                            "p (w c) -> p w c", c=D
                        ),
                        ogrp[:, :, 0:D],
                        r[:, 0:gl].unsqueeze(2).broadcast_to([128, gl, D]),
                    )
                    b = 2 * p + h
                    if p == NP - 1:
                        # last pair: per-group output DMA shortens the tail
                        nc.sync.dma_start(
                            out=outb[b * 128:(b + 1) * 128,
                                     g0 * D:(g0 + gl) * D],
                            in_=osb[h][:, g0 * D:(g0 + gl) * D],
                        )
                    elif gi == len(GRP) - 1:
                        nc.sync.dma_start(
                            out=outb[b * 128:(b + 1) * 128, :], in_=osb[h][:]
                        )

                # MM2 group gi covers windows [7gi, 7gi+7): ready once the
                # chunk holding its last window (and the back source) is
                # masked.  Stagger h1 one chunk later to smooth PE load.
                # Groups triggering at >= DEFER_AT run early in the NEXT pair.
                deferred, pending = pending, []
                trig = {}
                for gi, (g0, gl) in enumerate(GRP):
                    c_ready = (g0 + gl - 1) // 2
                    for h, c_t in ((0, c_ready), (1, c_ready + 1)):
                        c_t = min(c_t, NCH - 1)
                        if c_t >= DEFER_AT:
                            pending.append(
                                lambda h=h, gi=gi, f=mm2_group: f(h, gi)
                            )
                        else:
                            trig.setdefault(c_t, []).append((h, gi))

                for c in range(NCH):
                    S = psp.tile([128, 1024], F32, tag="S")
                    for ww in range(2):
                        w = 2 * c + ww
                        n1 = 256 if w < NW - 1 else 128
                        for h in range(2):
                            nc.tensor.matmul(
                                S[:, 512 * h + 256 * ww:
                                   512 * h + 256 * ww + n1],
                                lhsT=k_[64 * h:64 * h + 64, w * W:(w + 1) * W],
                                rhs=q_[64 * h:64 * h + 64, w * W: w * W + n1],
                                start=True, stop=True,
                            )
                    if c < NCH - 1:
                        nc.scalar.activation(
                            eh[:, c * 1024:(c + 1) * 1024], S[:, 0:1024],
                            mybir.ActivationFunctionType.Exp,
                            scale=float(D) ** -0.5,
                        )
                    else:
                        # last chunk: window 31 strip is 128 cols per half;
                        # one call over [0:896] also exps the unused junk in
                        # [384:512] (never read) to save a call.
                        nc.scalar.activation(
                            eh[:, c * 1024: c * 1024 + 896],
                            S[:, 0:896],
                            mybir.ActivationFunctionType.Exp,
                            scale=float(D) ** -0.5,
                        )
                    # causal mask on this chunk's 4 cur blocks (strided)
                    cur = eh[:, c * 1024:(c + 1) * 1024].rearrange(
                        "p (w x) -> p w x", x=256
                    )[:, :, 0:128]
                    nc.vector.tensor_mul(
                        cur,
                        cur,
                        m_t[:].unsqueeze(1).broadcast_to([128, 4, 128]),
                    )
                    for fn in deferred[c * 2:(c + 1) * 2]:
                        fn()  # previous pair's late MM2 groups
                    for h, gi in trig.get(c, ()):
                        mm2_group(h, gi)
            for fn in pending:  # flush the final pair's late groups
                fn()
    nc.finalize()
    return nc


def _mask():
    j = np.arange(128)[:, None]
    i = np.arange(128)[None, :]
    return (i >= j).astype(NPBF16)                     # [j, i] allowed mask


def _rope(x):
    # x: [B', N, D] f32; global-position angles
    inv = 1.0 / THETA ** (np.arange(0, D, 2, dtype=np.float32) / D)
    ang = np.arange(N, dtype=np.float32)[:, None] * inv[None, :]   # [N, 32]
    cos = np.cos(ang)
    sin = np.sin(ang)
    lo, hi = x[..., : D // 2], x[..., D // 2:]
    out = np.empty_like(x)
    out[..., : D // 2] = lo * cos - hi * sin
    out[..., D // 2:] = hi * cos + lo * sin
    return out


def kernel(q, k, v):
    if "nc" not in _CACHE:
        _CACHE["nc"] = _build_program()
    nc = _CACHE["nc"]
    m01 = _mask()

    qr = _rope(q)
    kr = _rope(k)

    in_maps = []
    for c in range(NCORES):
        sl = slice(c * BL, (c + 1) * BL)
        qc, kc, vc = qr[sl], kr[sl], v[sl]          # [16, N, 64] f32
        # d-major, batch pairs stacked on partitions: [NP, 2*64, N]
        qtc = qc.transpose(0, 2, 1).reshape(NP, 128, N)
        ktc = kc.transpose(0, 2, 1).reshape(NP, 128, N)
        # v blocked [16, 128, 32, 65] with ones column
        vbc = np.empty((BL, 128, NW, 65), dtype=NPBF16)
        vbc[..., :64] = vc.reshape(BL, NW, W, D).transpose(0, 2, 1, 3)
        vbc[..., 64] = 1.0
        in_maps.append({
            "qt": qtc.reshape(NP * 128, N).astype(NPBF16),
            "kt": ktc.reshape(NP * 128, N).astype(NPBF16),
            "vb": vbc.reshape(BL * 128, NW * 65),
            "m01": m01,
        })

    res = run_bass_kernel_spmd(nc, in_maps, list(range(NCORES)))
    _CACHE["last_results"] = res
    out = np.empty((B, N, D), dtype=np.float32)
    for c in range(NCORES):
        ob = res.results[c]["outb"].astype(np.float32).reshape(BL, 128, NW, D)
        out[c * BL:(c + 1) * BL] = (
            ob.transpose(0, 2, 1, 3).reshape(BL, N, D)
        )
    return out


if __name__ == "__main__":
    rng = np.random.default_rng(0)
    q = rng.standard_normal((B, N, D), dtype=np.float32)
    k = rng.standard_normal((B, N, D), dtype=np.float32)
    v = rng.standard_normal((B, N, D), dtype=np.float32)
    o = kernel(q, k, v)
    print("out", o.shape, o.dtype, np.abs(o).max())
